# revision 5
# baseline (speedup 1.0000x reference)
"""Trainium2 Bass kernel for nn_Critic GNN (NNConv x3 + global mean pool + MLP head).

Self-contained: hardcodes shapes; shards edges across 8 cores by dst-node range.

Algorithm per NNConv layer (per core, its ~E/8 edges, dst in its 500-node range):
  eh   = lrelu(edge_attr @ w1 + b1)                      [E, hid]   (PE, fp32)
  msg[e,o] = sum_{i,k} h[src_e,i] * eh[e,k] * w2[k,i,o] + h[src_e] @ B2
    computed as msgT[o, e] accumulated over (i, k-chunk) in PSUM:
      zT[(i,kc), e] = ehT[kc][k, e] * h_srcT[i, e]       (DVE bf16, broadcast of
        h row via PE transpose of a free-broadcast column AP)
      msgT += W2p[(i,kc) rows, o].T @ zT                 (PE bf16, N=512)
  agg[v,o] = sum_{e: dst=v} msg[e,o] / max(deg v,1)      one-hot is_equal matmul;
    1/deg folded into msg as a per-edge scale during PSUM->SBUF copy
  h' = lrelu(agg + h @ root + bias); h'' = concat(h', x)
  AllGather h'' across cores (each core owns 500 nodes)
Pool: per-core partial one-hot(batch) matmul + AllReduce + 1/graph-size scale.
Head: 3 small matmuls on every core; output read from core 0.
"""

import numpy as np
import ml_dtypes

import concourse.bass as bass
import concourse.bacc as bacc
import concourse.mybir as mybir
import concourse.tile as tile

F32 = mybir.dt.float32
BF16 = mybir.dt.bfloat16
I32 = mybir.dt.int32
AF = mybir.ActivationFunctionType
COPY = AF.Copy
LRELU = AF.Lrelu

N_NODES = 4000
N_EDGES = 12000
N_GRAPHS = 128
NODE_F = 16
EDGE_F = 4
SLOPE = 0.01
N_CORES = 8
V = N_NODES // N_CORES          # 500 nodes per core
VT = 4                          # v-tiles per core
VTS = V // VT                   # 125 nodes per v-tile

# layer dims: (in_feat, hidden, out)
LAYERS = [
    (NODE_F, 128, 128),
    (NODE_F + 128, 256, 128),
    (NODE_F + 128, 256, 256),
]
F1 = 128 + NODE_F   # h1cat width 144
F2 = 128 + NODE_F   # h2cat width 144
F3 = 256 + NODE_F   # h3cat width 272


def _build_nc(e_pad: int):
    nhalf = e_pad // 1024
    nech = e_pad // 128      # 128-edge chunks
    nc = bacc.Bacc(
        "TRN2", target_bir_lowering=False, debug=False,
        enable_asserts=False, num_devices=N_CORES,
    )

    # ---- inputs ----
    din = {}
    def inp(name, shape, dt=F32):
        din[name] = nc.dram_tensor(name, list(shape), dt, kind="ExternalInput")
        return din[name]

    eaT = inp("eaT", [EDGE_F, e_pad])
    src_idx = inp("src_idx", [e_pad, 1], I32)
    dstl = inp("dstl", [e_pad, 1])
    rc_dst = inp("rc_dst", [e_pad, 1])
    x_full = inp("x_full", [N_NODES, NODE_F])
    x_own = inp("x_own", [V, NODE_F])
    batch_own = inp("batch_own", [V, 1])
    rc_g = inp("rc_g", [N_GRAPHS, 1])
    ident_bf = inp("ident_bf", [128, 128], BF16)
    ident_f = inp("ident_f", [128, 128])
    iotaV = inp("iotaV", [128, 512])
    iotaG = inp("iotaG", [128, 128])
    ones_row = inp("ones_row", [1, 128])

    for li, (inf, hid, out) in enumerate(LAYERS):
        inp(f"w1_{li}", [EDGE_F, hid])
        inp(f"b1_{li}", [128, hid // 128])
        inp(f"w2p_{li}", [inf * hid, out], BF16)
        inp(f"b2_{li}", [inf, out], BF16)
        inp(f"root_{li}", [inf, out])
        inp(f"biasrow_{li}", [1, out])
    inp("fc1w", [F3, 256])
    inp("fc1b", [128, 2])
    inp("fc2w", [256, 512])
    inp("fc2b", [128, 4])
    inp("fc3w", [512, 1])
    inp("fc3b", [1, 1])

    y = nc.dram_tensor("y", [1, N_GRAPHS], F32, kind="ExternalOutput")

    RG = [list(range(N_CORES))]

    with tile.TileContext(nc) as tc:
        with (
            tc.tile_pool(name="sb", bufs=1) as sb,
            tc.tile_pool(name="sbw", bufs=2) as sbw,          # working tiles
            tc.tile_pool(name="ps_msg", bufs=1, space="PSUM") as ps_msg,
            tc.tile_pool(name="ps_b", bufs=2, space="PSUM") as ps_b,
            tc.tile_pool(name="ps_wk", bufs=2, space="PSUM") as ps_wk,
            tc.tile_pool(name="dram", bufs=1, space="DRAM") as dram,
        ):
            # ---------- load constants ----------
            identb_sb = sb.tile([128, 128], BF16)
            identf_sb = sb.tile([128, 128], F32)
            iotaV_sb = sb.tile([128, 512], F32)
            iotaG_sb = sb.tile([128, 128], F32)
            ones_sb = sb.tile([1, 128], F32)
            nc.sync.dma_start(identb_sb[:], ident_bf[:])
            nc.sync.dma_start(identf_sb[:], ident_f[:])
            nc.sync.dma_start(iotaV_sb[:], iotaV[:])
            nc.sync.dma_start(iotaG_sb[:], iotaG[:])
            nc.sync.dma_start(ones_sb[:], ones_row[:])

            eaT_sb = sb.tile([EDGE_F, e_pad], F32)
            nc.sync.dma_start(eaT_sb[:], eaT[:])
            srci_sb = sb.tile([128, nech], I32)
            dstl_sb = sb.tile([128, nech], F32)
            rcd_sb = sb.tile([128, nech], F32)
            nc.sync.dma_start(srci_sb[:], src_idx.rearrange("(c p) o -> p (c o)", p=128))
            nc.sync.dma_start(dstl_sb[:], dstl.rearrange("(c p) o -> p (c o)", p=128))
            nc.sync.dma_start(rcd_sb[:], rc_dst.rearrange("(c p) o -> p (c o)", p=128))

            # one-hot DT[e, v] chunks (shared across layers): [128, nech*500]
            DT_sb = sb.tile([128, nech * V], F32)
            for c in range(nech):
                nc.vector.tensor_scalar(
                    out=DT_sb[:, c * V:(c + 1) * V], in0=iotaV_sb[:, :V],
                    scalar1=dstl_sb[:, c:c + 1], scalar2=None,
                    op0=mybir.AluOpType.is_equal,
                )

            # small per-layer weights to SBUF
            lw = []
            for li, (inf, hid, out) in enumerate(LAYERS):
                nit = (inf + 127) // 128      # in chunks
                d = {}
                d["w1"] = sb.tile([EDGE_F, hid], F32, name=f"w1sb{li}")
                nc.sync.dma_start(d["w1"][:], din[f"w1_{li}"][:])
                d["b1"] = sb.tile([128, hid // 128], F32, name=f"b1sb{li}")
                nc.sync.dma_start(d["b1"][:], din[f"b1_{li}"][:])
                d["b2"] = sb.tile([128, nit, out], BF16, name=f"b2sb{li}")
                d["root"] = sb.tile([128, nit, out], F32, name=f"rootsb{li}")
                for it in range(nit):
                    p = min(128, inf - it * 128)
                    nc.sync.dma_start(d["b2"][:p, it, :], din[f"b2_{li}"][it * 128:it * 128 + p, :])
                    nc.sync.dma_start(d["root"][:p, it, :], din[f"root_{li}"][it * 128:it * 128 + p, :])
                d["biasrow"] = sb.tile([1, out], F32, name=f"brsb{li}")
                nc.sync.dma_start(d["biasrow"][:], din[f"biasrow_{li}"][:])
                lw.append(d)

            fc1w_sb = sb.tile([128, 3, 256], F32)
            for it in range(3):
                p = min(128, F3 - it * 128)
                nc.sync.dma_start(fc1w_sb[:p, it, :], din["fc1w"][it * 128:it * 128 + p, :])
            fc2w_sb = sb.tile([128, 2, 512], F32)
            for it in range(2):
                nc.sync.dma_start(fc2w_sb[:, it, :], din["fc2w"][it * 128:(it + 1) * 128, :])
            fc3w_sb = sb.tile([128, 4, 1], F32)
            for it in range(4):
                nc.sync.dma_start(fc3w_sb[:, it, :], din["fc3w"][it * 128:(it + 1) * 128, :])
            fc1b_sb = sb.tile([128, 2], F32)
            nc.sync.dma_start(fc1b_sb[:], din["fc1b"][:])
            fc2b_sb = sb.tile([128, 4], F32)
            nc.sync.dma_start(fc2b_sb[:], din["fc2b"][:])
            fc3b_sb = sb.tile([1, 1], F32)
            nc.sync.dma_start(fc3b_sb[:], din["fc3b"][:])

            # DRAM intermediates for h layers (AllGather)
            h_own = [
                dram.tile([V, F1], F32, name="h0own"),
                dram.tile([V, F2], F32, name="h1own"),
            ]
            h_full = [
                dram.tile([N_NODES, F1], F32, name="h0full", addr_space="Shared"),
                dram.tile([N_NODES, F2], F32, name="h1full", addr_space="Shared"),
            ]
            pool_in = dram.tile([N_GRAPHS, F3], F32)
            pool_out = dram.tile([N_GRAPHS, F3], F32, addr_space="Shared")

            hcat_tiles = None

            for li, (inf, hid, out) in enumerate(LAYERS):
                d = lw[li]
                nkt = hid // 128
                nit = (inf + 127) // 128
                not_ = out // 128
                gather_src = x_full if li == 0 else h_full[li - 1]

                # ---------- phase A: ehT [hid, e_pad] bf16 ----------
                ehT = sbw.tile([128, nkt, e_pad], BF16, tag="ehT", name=f"ehT{li}", bufs=1)
                for kt in range(nkt):
                    for ec in range(e_pad // 512):
                        ehp = ps_wk.tile([128, 512], F32, tag="wk", name=f"ehp{li}")
                        nc.tensor.matmul(
                            ehp[:], lhsT=d["w1"][:, kt * 128:(kt + 1) * 128],
                            rhs=eaT_sb[:, ec * 512:(ec + 1) * 512],
                            start=True, stop=True,
                        )
                        nc.scalar.activation(
                            ehT[:, kt, ec * 512:(ec + 1) * 512], ehp[:],
                            LRELU, bias=d["b1"][:, kt:kt + 1], alpha=SLOPE,
                        )

                # ---------- phase B: gather h_src, cast bf16, h_srcT ----------
                hs_bf = sbw.tile([128, nech, inf], BF16, tag="hsbf", name=f"hsbf{li}", bufs=1)
                hsT = sbw.tile([128, nit, e_pad], BF16, tag="hsT", name=f"hsT{li}", bufs=1)
                for c in range(nech):
                    hsg = sbw.tile([128, inf], F32, tag="hsg", name=f"hsg{li}", bufs=2)
                    nc.gpsimd.indirect_dma_start(
                        out=hsg[:], out_offset=None, in_=gather_src[:, :],
                        in_offset=bass.IndirectOffsetOnAxis(ap=srci_sb[:, c:c + 1], axis=0),
                    )
                    nc.vector.tensor_copy(hs_bf[:, c, :], hsg[:])
                    for it in range(nit):
                        p = min(128, inf - it * 128)
                        hstp = ps_wk.tile([128, 512], BF16, tag="wk", name=f"hstp{li}")
                        nc.tensor.transpose(
                            hstp[:p, :128],
                            hs_bf[:, c, it * 128:it * 128 + p], identb_sb[:],
                        )
                        nc.scalar.activation(
                            hsT[:p, it, c * 128:(c + 1) * 128], hstp[:p, :128], COPY,
                        )

                # msg scaled output tiles [128, nech*out] f32
                msg_sb = sbw.tile([128, nech, out], F32, tag="msg_sb", name=f"msg{li}", bufs=1)

                # ---------- phase C: big contraction per half ----------
                for hf in range(nhalf):
                    e0 = hf * 1024
                    msps = [
                        ps_msg.tile([128, 1024], F32, tag=f"msg{ot}", name=f"msps{li}_{ot}")
                        for ot in range(not_)
                    ]
                    # B2 bias term: msgT += B2[it].T @ h_srcT[it]
                    for ot in range(not_):
                        for q in range(2):
                            for it in range(nit):
                                p = min(128, inf - it * 128)
                                nc.tensor.matmul(
                                    msps[ot][:, q * 512:(q + 1) * 512],
                                    lhsT=d["b2"][:p, it, ot * 128:(ot + 1) * 128],
                                    rhs=hsT[:p, it, e0 + q * 512:e0 + (q + 1) * 512],
                                    start=(it == 0), stop=False,
                                    skip_group_check=True,
                                )
                    for i in range(inf):
                        bps = ps_b.tile([128, 1024], BF16, tag="bps", name=f"bps{li}")
                        for c8 in range(8):
                            c = hf * 8 + c8
                            nc.tensor.transpose(
                                bps[:, c8 * 128:(c8 + 1) * 128],
                                hs_bf[:, c, i:i + 1].to_broadcast([128, 128]),
                                identb_sb[:],
                            )
                        bsb = sbw.tile([128, 1024], BF16, tag="bsb", name=f"bsb{li}", bufs=2)
                        nc.scalar.activation(bsb[:], bps[:], COPY)
                        for kc in range(nkt):
                            zt = sbw.tile([128, 1024], BF16, tag="zt", name=f"zt{li}", bufs=3)
                            nc.vector.tensor_tensor(
                                out=zt[:], in0=ehT[:, kc, e0:e0 + 1024], in1=bsb[:],
                                op=mybir.AluOpType.mult,
                            )
                            w2t = sbw.tile([128, out], BF16, tag="w2t", name=f"w2t{li}", bufs=3)
                            r0 = (i * hid + kc * 128)
                            nc.sync.dma_start(w2t[:], din[f"w2p_{li}"][r0:r0 + 128, :])
                            last = (i == inf - 1) and (kc == nkt - 1)
                            for ot in range(not_):
                                for q in range(2):
                                    nc.tensor.matmul(
                                        msps[ot][:, q * 512:(q + 1) * 512],
                                        lhsT=w2t[:, ot * 128:(ot + 1) * 128],
                                        rhs=zt[:, q * 512:(q + 1) * 512],
                                        start=False, stop=last,
                                        skip_group_check=True,
                                    )
                    # ---------- phase D: msgT -> msg (transpose + rc scale) ----------
                    for ot in range(not_):
                        mts = sbw.tile([128, 1024], F32, tag="mts", name=f"mts{li}", bufs=2)
                        nc.scalar.activation(mts[:], msps[ot][:], COPY)
                        for c8 in range(8):
                            c = hf * 8 + c8
                            mtp = ps_wk.tile([128, 512], F32, tag="wk", name=f"mtp{li}")
                            nc.tensor.transpose(
                                mtp[:, :128], mts[:, c8 * 128:(c8 + 1) * 128], identf_sb[:],
                            )
                            nc.scalar.activation(
                                msg_sb[:, c, ot * 128:(ot + 1) * 128], mtp[:, :128],
                                COPY, scale=rcd_sb[:, c:c + 1],
                            )

                # hT_own for root term: transpose own h rows
                # layer0: own h = x_own; else hcat_tiles from previous layer
                hTo = sbw.tile([128, nit, V], F32, tag="hTo", name=f"hTo{li}", bufs=1)
                for vt in range(VT):
                    for it in range(nit):
                        p = min(128, inf - it * 128)
                        htp = ps_wk.tile([128, 512], F32, tag="wk", name=f"htp{li}")
                        if li == 0:
                            hsrc_own = sbw.tile([128, NODE_F], F32, tag="hso", name=f"hso{li}", bufs=2)
                            nc.sync.dma_start(
                                hsrc_own[:VTS, :], x_own[vt * VTS:(vt + 1) * VTS, :]
                            )
                            nc.tensor.transpose(
                                htp[:p, :VTS], hsrc_own[:VTS, it * 128:it * 128 + p],
                                identf_sb[:VTS, :VTS],
                            )
                        else:
                            nc.tensor.transpose(
                                htp[:p, :VTS],
                                hcat_tiles[vt][:VTS, it * 128:it * 128 + p],
                                identf_sb[:VTS, :VTS],
                            )
                        nc.scalar.activation(
                            hTo[:p, it, vt * VTS:(vt + 1) * VTS], htp[:p, :VTS], COPY,
                        )

                # ---------- phase E: aggregation + h' ----------
                new_hcat = []
                catw = out + NODE_F
                for vt in range(VT):
                    aggp = ps_wk.tile([128, 512], F32, tag="wk", name=f"aggp{li}")
                    for c in range(nech):
                        nc.tensor.matmul(
                            aggp[:VTS, :out],
                            lhsT=DT_sb[:, c * V + vt * VTS: c * V + (vt + 1) * VTS],
                            rhs=msg_sb[:, c, :],
                            start=(c == 0), stop=False,
                            skip_group_check=True,
                        )
                    for it in range(nit):
                        p = min(128, inf - it * 128)
                        nc.tensor.matmul(
                            aggp[:VTS, :out],
                            lhsT=hTo[:p, it, vt * VTS:(vt + 1) * VTS],
                            rhs=d["root"][:p, it, :],
                            start=False, stop=False,
                            skip_group_check=True,
                        )
                    nc.tensor.matmul(
                        aggp[:VTS, :out],
                        lhsT=ones_sb[:1, :VTS], rhs=d["biasrow"][:1, :],
                        start=False, stop=True,
                        skip_group_check=True,
                    )
                    hcat = sbw.tile([128, catw], F32, tag=f"hcat{li % 2}_{vt}", name=f"hcat{li}_{vt}", bufs=1)
                    nc.scalar.activation(hcat[:VTS, :out], aggp[:VTS, :out], LRELU, alpha=SLOPE)
                    nc.sync.dma_start(
                        hcat[:VTS, out:catw], x_own[vt * VTS:(vt + 1) * VTS, :]
                    )
                    new_hcat.append(hcat)
                hcat_tiles = new_hcat

                # ---------- phase F: write h_own + AllGather ----------
                if li < 2:
                    for vt in range(VT):
                        nc.sync.dma_start(
                            h_own[li][vt * VTS:(vt + 1) * VTS, :], hcat_tiles[vt][:VTS, :catw]
                        )
                    nc.gpsimd.collective_compute(
                        "AllGather", mybir.AluOpType.bypass,
                        replica_groups=RG,
                        ins=[h_own[li].opt()], outs=[h_full[li].opt()],
                    )

            # ---------- pooling ----------
            poolp = ps_wk.tile([128, 512], F32, tag="wk", name="poolp")
            for vt in range(VT):
                pt = sbw.tile([128, 128], F32, tag="pt", name="pt", bufs=2)
                bo = sbw.tile([128, 1], F32, tag="bo", name="bo", bufs=2)
                nc.sync.dma_start(bo[:VTS, :], batch_own[vt * VTS:(vt + 1) * VTS, :])
                nc.vector.tensor_scalar(
                    out=pt[:VTS, :], in0=iotaG_sb[:VTS, :],
                    scalar1=bo[:VTS, :1], scalar2=None,
                    op0=mybir.AluOpType.is_equal,
                )
                nc.tensor.matmul(
                    poolp[:N_GRAPHS, :F3],
                    lhsT=pt[:VTS, :N_GRAPHS], rhs=hcat_tiles[vt][:VTS, :F3],
                    start=(vt == 0), stop=(vt == VT - 1),
                    skip_group_check=True,
                )
            pool_sb = sbw.tile([128, F3], F32, name="pool_sb", bufs=1)
            nc.scalar.activation(pool_sb[:N_GRAPHS, :], poolp[:N_GRAPHS, :F3], COPY)
            nc.sync.dma_start(pool_in[:], pool_sb[:N_GRAPHS, :])
            nc.gpsimd.collective_compute(
                "AllReduce", mybir.AluOpType.add,
                replica_groups=RG,
                ins=[pool_in.opt()], outs=[pool_out.opt()],
            )
            pooled = sbw.tile([128, F3], F32, name="pooled", bufs=1)
            nc.sync.dma_start(pooled[:N_GRAPHS, :], pool_out[:])
            rcg_sb = sbw.tile([128, 1], F32, name="rcg_sb", bufs=1)
            nc.sync.dma_start(rcg_sb[:N_GRAPHS, :], rc_g[:])
            nc.vector.tensor_scalar_mul(pooled[:N_GRAPHS, :], pooled[:N_GRAPHS, :], rcg_sb[:N_GRAPHS, :1])

            # pooledT [F3 -> 3 chunks of 128, 128g]
            pooledT = sbw.tile([128, 3, 128], F32, name="pooledT", bufs=1)
            for it in range(3):
                p = min(128, F3 - it * 128)
                ptp = ps_wk.tile([128, 512], F32, tag="wk", name="ptp")
                nc.tensor.transpose(
                    ptp[:p, :N_GRAPHS], pooled[:N_GRAPHS, it * 128:it * 128 + p], identf_sb[:],
                )
                nc.scalar.activation(pooledT[:p, it, :], ptp[:p, :N_GRAPHS], COPY)

            # ---------- head ----------
            z1T = sbw.tile([128, 2, 128], F32, name="z1T", bufs=1)
            for mt in range(2):
                zp = ps_wk.tile([128, 512], F32, tag="wk", name="z1p")
                for it in range(3):
                    p = min(128, F3 - it * 128)
                    nc.tensor.matmul(
                        zp[:, :N_GRAPHS],
                        lhsT=fc1w_sb[:p, it, mt * 128:(mt + 1) * 128],
                        rhs=pooledT[:p, it, :],
                        start=(it == 0), stop=(it == 2),
                        skip_group_check=True,
                    )
                nc.scalar.activation(
                    z1T[:, mt, :], zp[:, :N_GRAPHS], LRELU,
                    bias=fc1b_sb[:, mt:mt + 1], alpha=SLOPE,
                )
            z2T = sbw.tile([128, 4, 128], F32, name="z2T", bufs=1)
            for mt in range(4):
                zp = ps_wk.tile([128, 512], F32, tag="wk", name="z2p")
                for it in range(2):
                    nc.tensor.matmul(
                        zp[:, :N_GRAPHS],
                        lhsT=fc2w_sb[:, it, mt * 128:(mt + 1) * 128],
                        rhs=z1T[:, it, :],
                        start=(it == 0), stop=(it == 1),
                        skip_group_check=True,
                    )
                nc.scalar.activation(
                    z2T[:, mt, :], zp[:, :N_GRAPHS], LRELU,
                    bias=fc2b_sb[:, mt:mt + 1], alpha=SLOPE,
                )
            yp = ps_wk.tile([128, 512], F32, tag="wk", name="yp")
            for it in range(4):
                nc.tensor.matmul(
                    yp[:1, :N_GRAPHS],
                    lhsT=fc3w_sb[:, it, :], rhs=z2T[:, it, :],
                    start=(it == 0), stop=False,
                    skip_group_check=True,
                )
            nc.tensor.matmul(
                yp[:1, :N_GRAPHS],
                lhsT=fc3b_sb[:1, :1], rhs=ones_sb[:1, :N_GRAPHS],
                start=False, stop=True,
                skip_group_check=True,
            )
            y_sb = sbw.tile([1, N_GRAPHS], F32, name="y_sb", bufs=1)
            nc.scalar.activation(y_sb[:], yp[:1, :N_GRAPHS], COPY)
            nc.sync.dma_start(y[:], y_sb[:])

    nc.compile()
    return nc


# ---------------- host side ----------------

_CACHE = {}


def _prep_inputs(x, edge_index, edge_attr, batch, weights):
    """Returns (e_pad, in_maps list of dicts)."""
    x = np.asarray(x, np.float32)
    ei = np.asarray(edge_index).astype(np.int64)
    ea = np.asarray(edge_attr, np.float32)
    batch = np.asarray(batch).astype(np.int64)

    src, dst = ei[0], ei[1]
    cnt = np.bincount(dst, minlength=N_NODES).astype(np.float32)
    rc_edge = 1.0 / np.maximum(cnt, 1.0)      # per dst node
    gcnt = np.bincount(batch, minlength=N_GRAPHS).astype(np.float32)
    rc_g = (1.0 / np.maximum(gcnt, 1.0)).reshape(N_GRAPHS, 1)

    bins = (dst // V).astype(np.int64)
    order = np.argsort(bins, kind="stable")
    counts = np.bincount(bins, minlength=N_CORES)
    e_pad = max(1024, int(np.ceil(counts.max() / 1024)) * 1024)

    iotaV = np.broadcast_to(np.arange(512, dtype=np.float32), (128, 512)).copy()
    iotaG = np.broadcast_to(np.arange(128, dtype=np.float32), (128, 128)).copy()
    ident_f = np.eye(128, dtype=np.float32)
    ident_bf = ident_f.astype(ml_dtypes.bfloat16)
    ones_row = np.ones((1, 128), np.float32)

    common = dict(
        x_full=x, rc_g=rc_g, ident_bf=ident_bf, ident_f=ident_f,
        iotaV=iotaV, iotaG=iotaG, ones_row=ones_row,
    )
    for li, (inf, hid, out) in enumerate(LAYERS):
        w1, b1, w2, b2, root, bias = weights[li]
        common[f"w1_{li}"] = np.asarray(w1, np.float32)
        common[f"b1_{li}"] = np.asarray(b1, np.float32).reshape(hid // 128, 128).T.copy()
        w2p = (
            np.asarray(w2, np.float32)
            .reshape(hid, inf, out).transpose(1, 0, 2).reshape(inf * hid, out)
        )
        common[f"w2p_{li}"] = w2p.astype(ml_dtypes.bfloat16)
        common[f"b2_{li}"] = np.asarray(b2, np.float32).reshape(inf, out).astype(ml_dtypes.bfloat16)
        common[f"root_{li}"] = np.asarray(root, np.float32)
        common[f"biasrow_{li}"] = np.asarray(bias, np.float32).reshape(1, out)
    fc1w, fc1b, fc2w, fc2b, fc3w, fc3b = weights[3]
    common["fc1w"] = np.asarray(fc1w, np.float32)
    common["fc1b"] = np.asarray(fc1b, np.float32).reshape(2, 128).T.copy()
    common["fc2w"] = np.asarray(fc2w, np.float32)
    common["fc2b"] = np.asarray(fc2b, np.float32).reshape(4, 128).T.copy()
    common["fc3w"] = np.asarray(fc3w, np.float32).reshape(512, 1)
    common["fc3b"] = np.asarray(fc3b, np.float32).reshape(1, 1)

    in_maps = []
    for c in range(N_CORES):
        sel = order[bins[order] == c]
        n = len(sel)
        srcp = np.zeros(e_pad, np.int32)
        dstlp = np.full(e_pad, -1.0, np.float32)
        rcp = np.zeros(e_pad, np.float32)
        eap = np.zeros((e_pad, EDGE_F), np.float32)
        srcp[:n] = src[sel]
        dstlp[:n] = (dst[sel] - c * V).astype(np.float32)
        rcp[:n] = rc_edge[dst[sel]]
        eap[:n] = ea[sel]
        m = dict(common)
        m["eaT"] = np.ascontiguousarray(eap.T)
        m["src_idx"] = srcp.reshape(e_pad, 1)
        m["dstl"] = dstlp.reshape(e_pad, 1)
        m["rc_dst"] = rcp.reshape(e_pad, 1)
        m["x_own"] = np.ascontiguousarray(x[c * V:(c + 1) * V])
        m["batch_own"] = batch[c * V:(c + 1) * V].astype(np.float32).reshape(V, 1)
        in_maps.append(m)
    return e_pad, in_maps


class _Runner:
    """Persistent PJRT executable for repeated timed runs."""

    def __init__(self, nc):
        import jax
        from jax.sharding import Mesh, PartitionSpec
        from jax.experimental.shard_map import shard_map
        from concourse import bass2jax

        bass2jax.install_neuronx_cc_hook()
        self.nc = nc
        partition_name = (
            nc.partition_id_tensor.name if nc.partition_id_tensor else None
        )
        in_names, out_names, out_avals, zero_outs = [], [], [], []
        for alloc in nc.m.functions[0].allocations:
            if not isinstance(alloc, mybir.MemoryLocationSet):
                continue
            name = alloc.memorylocations[0].name
            if alloc.kind == "ExternalInput":
                if name == partition_name:
                    continue
                in_names.append(name)
            elif alloc.kind == "ExternalOutput":
                out_names.append(name)
                shape = tuple(alloc.tensor_shape)
                dtype = mybir.dt.np(alloc.dtype)
                out_avals.append(jax.core.ShapedArray(shape, dtype))
                zero_outs.append(np.zeros(shape, dtype))
        self.in_names = in_names
        self.out_names = out_names
        self.zero_outs = zero_outs
        n_params = len(in_names)
        n_outs = len(out_names)
        all_names = in_names + out_names
        if partition_name is not None:
            all_names = all_names + [partition_name]
        donate = tuple(range(n_params, n_params + n_outs))

        def _body(*args):
            operands = list(args)
            if partition_name is not None:
                operands.append(bass2jax.partition_id_tensor())
            outs = bass2jax._bass_exec_p.bind(
                *operands,
                out_avals=tuple(out_avals),
                in_names=tuple(all_names),
                out_names=tuple(out_names),
                lowering_input_output_aliases=(),
                sim_require_finite=False,
                sim_require_nnan=False,
                nc=nc,
            )
            return tuple(outs)

        devices = jax.devices()[:N_CORES]
        mesh = Mesh(np.asarray(devices), ("core",))
        in_specs = (PartitionSpec("core"),) * (n_params + n_outs)
        out_specs = (PartitionSpec("core"),) * n_outs
        self.fn = jax.jit(
            shard_map(_body, mesh=mesh, in_specs=in_specs,
                      out_specs=out_specs, check_rep=False),
            donate_argnums=donate, keep_unused=True,
        )
        self.out_avals = out_avals

    def __call__(self, in_maps):
        concat_in = [
            np.concatenate([np.asarray(in_maps[c][n]) for c in range(N_CORES)], axis=0)
            for n in self.in_names
        ]
        concat_zero = [
            np.zeros((N_CORES * z.shape[0], *z.shape[1:]), z.dtype)
            for z in self.zero_outs
        ]
        out = self.fn(*concat_in, *concat_zero)
        return {
            n: np.asarray(out[i]).reshape(N_CORES, *self.out_avals[i].shape)[0]
            for i, n in enumerate(self.out_names)
        }


def _get_runner(e_pad):
    if e_pad not in _CACHE:
        nc = _build_nc(e_pad)
        _CACHE[e_pad] = _Runner(nc)
    return _CACHE[e_pad]


def kernel(x, edge_index, edge_attr, batch,
           m1w1, m1b1, m1w2, m1b2, root1, bias1,
           m2w1, m2b1, m2w2, m2b2, root2, bias2,
           m3w1, m3b1, m3w2, m3b2, root3, bias3,
           fc1w, fc1b, fc2w, fc2b, fc3w, fc3b):
    weights = [
        (m1w1, m1b1, m1w2, m1b2, root1, bias1),
        (m2w1, m2b1, m2w2, m2b2, root2, bias2),
        (m3w1, m3b1, m3w2, m3b2, root3, bias3),
        (fc1w, fc1b, fc2w, fc2b, fc3w, fc3b),
    ]
    e_pad, in_maps = _prep_inputs(x, edge_index, edge_attr, batch, weights)
    runner = _get_runner(e_pad)
    out = runner(in_maps)
    return np.ascontiguousarray(out["y"].reshape(N_GRAPHS, 1)).astype(np.float32)


# revision 6
# speedup vs baseline: 28.4612x; 28.4612x over previous
"""Trainium2 Bass kernel for nn_Critic GNN (NNConv x3 + global mean pool + MLP head).

Self-contained: hardcodes shapes; shards edges across 8 cores by dst-node range.

Algorithm per NNConv layer (per core, its ~E/8 edges, dst in its 500-node range):
  eh   = lrelu(edge_attr @ w1 + b1)                      [E, hid]   (PE, fp32)
  msg[e,o] = sum_{i,k} h[src_e,i] * eh[e,k] * w2[k,i,o] + h[src_e] @ B2
    computed as msgT[o, e] accumulated over (i, k-chunk) in PSUM:
      zT[(i,kc), e] = ehT[kc][k, e] * h_srcT[i, e]       (DVE bf16, broadcast of
        h row via PE transpose of a free-broadcast column AP)
      msgT += W2p[(i,kc) rows, o].T @ zT                 (PE bf16, N=512)
  agg[v,o] = sum_{e: dst=v} msg[e,o] / max(deg v,1)      one-hot is_equal matmul;
    1/deg folded into msg as a per-edge scale during PSUM->SBUF copy
  h' = lrelu(agg + h @ root + bias); h'' = concat(h', x)
  AllGather h'' across cores (each core owns 500 nodes)
Pool: per-core partial one-hot(batch) matmul + AllReduce + 1/graph-size scale.
Head: 3 small matmuls on every core; output read from core 0.
"""

import numpy as np
import ml_dtypes

import concourse.bass as bass
import concourse.bacc as bacc
import concourse.mybir as mybir
import concourse.tile as tile

F32 = mybir.dt.float32
BF16 = mybir.dt.bfloat16
I32 = mybir.dt.int32
AF = mybir.ActivationFunctionType
COPY = AF.Copy
LRELU = AF.Lrelu

N_NODES = 4000
N_EDGES = 12000
N_GRAPHS = 128
NODE_F = 16
EDGE_F = 4
SLOPE = 0.01
N_CORES = 8
V = N_NODES // N_CORES          # 500 nodes per core
VT = 4                          # v-tiles per core
VTS = V // VT                   # 125 nodes per v-tile

# layer dims: (in_feat, hidden, out)
LAYERS = [
    (NODE_F, 128, 128),
    (NODE_F + 128, 256, 128),
    (NODE_F + 128, 256, 256),
]
F1 = 128 + NODE_F   # h1cat width 144
F2 = 128 + NODE_F   # h2cat width 144
F3 = 256 + NODE_F   # h3cat width 272


def _build_nc(e_pad: int):
    nhalf = e_pad // 1024
    nech = e_pad // 128      # 128-edge chunks
    nc = bacc.Bacc(
        "TRN2", target_bir_lowering=False, debug=False,
        enable_asserts=False, num_devices=N_CORES,
    )

    # ---- inputs ----
    din = {}
    def inp(name, shape, dt=F32):
        din[name] = nc.dram_tensor(name, list(shape), dt, kind="ExternalInput")
        return din[name]

    eaT = inp("eaT", [EDGE_F, e_pad])
    src_idx = inp("src_idx", [e_pad, 1], I32)
    dstl = inp("dstl", [e_pad, 1])
    rc_dst = inp("rc_dst", [e_pad, 1])
    x_full = inp("x_full", [N_NODES, NODE_F])
    x_own = inp("x_own", [V, NODE_F])
    batch_own = inp("batch_own", [V, 1])
    rc_g = inp("rc_g", [N_GRAPHS, 1])
    ident_bf = inp("ident_bf", [128, 128], BF16)
    ident_f = inp("ident_f", [128, 128])
    iotaV = inp("iotaV", [128, 512])
    iotaG = inp("iotaG", [128, 128])
    ones_row = inp("ones_row", [1, 128])

    for li, (inf, hid, out) in enumerate(LAYERS):
        inp(f"w1_{li}", [EDGE_F, hid])
        inp(f"b1_{li}", [128, hid // 128])
        inp(f"w2p_{li}", [inf * hid, out], BF16)
        inp(f"b2_{li}", [inf, out], BF16)
        inp(f"root_{li}", [inf, out])
        inp(f"biasrow_{li}", [1, out])
    inp("fc1w", [F3, 256])
    inp("fc1b", [128, 2])
    inp("fc2w", [256, 512])
    inp("fc2b", [128, 4])
    inp("fc3w", [512, 1])
    inp("fc3b", [1, 1])

    y = nc.dram_tensor("y", [1, N_GRAPHS], F32, kind="ExternalOutput")

    RG = [list(range(N_CORES))]

    with tile.TileContext(nc) as tc:
        with (
            tc.tile_pool(name="sb", bufs=1) as sb,
            tc.tile_pool(name="sbw", bufs=2) as sbw,          # working tiles
            tc.tile_pool(name="ps_msg", bufs=1, space="PSUM") as ps_msg,
            tc.tile_pool(name="ps_b", bufs=2, space="PSUM") as ps_b,
            tc.tile_pool(name="ps_wk", bufs=2, space="PSUM") as ps_wk,
            tc.tile_pool(name="dram", bufs=1, space="DRAM") as dram,
        ):
            # ---------- load constants ----------
            identb_sb = sb.tile([128, 128], BF16)
            identf_sb = sb.tile([128, 128], F32)
            iotaV_sb = sb.tile([128, 512], F32)
            iotaG_sb = sb.tile([128, 128], F32)
            ones_sb = sb.tile([1, 128], F32)
            nc.sync.dma_start(identb_sb[:], ident_bf[:])
            nc.sync.dma_start(identf_sb[:], ident_f[:])
            nc.sync.dma_start(iotaV_sb[:], iotaV[:])
            nc.sync.dma_start(iotaG_sb[:], iotaG[:])
            nc.sync.dma_start(ones_sb[:], ones_row[:])

            eaT_sb = sb.tile([EDGE_F, e_pad], F32)
            nc.sync.dma_start(eaT_sb[:], eaT[:])
            srci_sb = sb.tile([128, nech], I32)
            dstl_sb = sb.tile([128, nech], F32)
            rcd_sb = sb.tile([128, nech], F32)
            nc.sync.dma_start(srci_sb[:], src_idx.rearrange("(c p) o -> p (c o)", p=128))
            nc.sync.dma_start(dstl_sb[:], dstl.rearrange("(c p) o -> p (c o)", p=128))
            nc.sync.dma_start(rcd_sb[:], rc_dst.rearrange("(c p) o -> p (c o)", p=128))

            # one-hot DT[e, v] chunks (shared across layers): [128, nech*500]
            DT_sb = sb.tile([128, nech * V], F32)
            for c in range(nech):
                nc.vector.tensor_scalar(
                    out=DT_sb[:, c * V:(c + 1) * V], in0=iotaV_sb[:, :V],
                    scalar1=dstl_sb[:, c:c + 1], scalar2=None,
                    op0=mybir.AluOpType.is_equal,
                )

            # small per-layer weights to SBUF
            lw = []
            for li, (inf, hid, out) in enumerate(LAYERS):
                nit = (inf + 127) // 128      # in chunks
                d = {}
                d["w1"] = sb.tile([EDGE_F, hid], F32, name=f"w1sb{li}")
                nc.sync.dma_start(d["w1"][:], din[f"w1_{li}"][:])
                d["b1"] = sb.tile([128, hid // 128], F32, name=f"b1sb{li}")
                nc.sync.dma_start(d["b1"][:], din[f"b1_{li}"][:])
                d["b2"] = sb.tile([128, nit, out], BF16, name=f"b2sb{li}")
                d["root"] = sb.tile([128, nit, out], F32, name=f"rootsb{li}")
                for it in range(nit):
                    p = min(128, inf - it * 128)
                    nc.sync.dma_start(d["b2"][:p, it, :], din[f"b2_{li}"][it * 128:it * 128 + p, :])
                    nc.sync.dma_start(d["root"][:p, it, :], din[f"root_{li}"][it * 128:it * 128 + p, :])
                d["biasrow"] = sb.tile([1, out], F32, name=f"brsb{li}")
                nc.sync.dma_start(d["biasrow"][:], din[f"biasrow_{li}"][:])
                lw.append(d)

            fc1w_sb = sb.tile([128, 3, 256], F32)
            for it in range(3):
                p = min(128, F3 - it * 128)
                nc.sync.dma_start(fc1w_sb[:p, it, :], din["fc1w"][it * 128:it * 128 + p, :])
            fc2w_sb = sb.tile([128, 2, 512], F32)
            for it in range(2):
                nc.sync.dma_start(fc2w_sb[:, it, :], din["fc2w"][it * 128:(it + 1) * 128, :])
            fc3w_sb = sb.tile([128, 4, 1], F32)
            for it in range(4):
                nc.sync.dma_start(fc3w_sb[:, it, :], din["fc3w"][it * 128:(it + 1) * 128, :])
            fc1b_sb = sb.tile([128, 2], F32)
            nc.sync.dma_start(fc1b_sb[:], din["fc1b"][:])
            fc2b_sb = sb.tile([128, 4], F32)
            nc.sync.dma_start(fc2b_sb[:], din["fc2b"][:])
            fc3b_sb = sb.tile([1, 1], F32)
            nc.sync.dma_start(fc3b_sb[:], din["fc3b"][:])

            # DRAM intermediates for h layers (AllGather)
            h_own = [
                dram.tile([V, F1], F32, name="h0own"),
                dram.tile([V, F2], F32, name="h1own"),
            ]
            h_full = [
                dram.tile([N_NODES, F1], F32, name="h0full", addr_space="Shared"),
                dram.tile([N_NODES, F2], F32, name="h1full", addr_space="Shared"),
            ]
            pool_in = dram.tile([N_GRAPHS, F3], F32)
            pool_out = dram.tile([N_GRAPHS, F3], F32, addr_space="Shared")

            hcat_tiles = None

            for li, (inf, hid, out) in enumerate(LAYERS):
                d = lw[li]
                nkt = hid // 128
                nit = (inf + 127) // 128
                not_ = out // 128
                gather_src = x_full if li == 0 else h_full[li - 1]

                # ---------- phase A: ehT [hid, e_pad] bf16 ----------
                ehT = sbw.tile([128, nkt, e_pad], BF16, tag="ehT", name=f"ehT{li}", bufs=1)
                for kt in range(nkt):
                    for ec in range(e_pad // 512):
                        ehp = ps_wk.tile([128, 512], F32, tag="wk", name=f"ehp{li}")
                        nc.tensor.matmul(
                            ehp[:], lhsT=d["w1"][:, kt * 128:(kt + 1) * 128],
                            rhs=eaT_sb[:, ec * 512:(ec + 1) * 512],
                            start=True, stop=True,
                        )
                        nc.scalar.activation(
                            ehT[:, kt, ec * 512:(ec + 1) * 512], ehp[:],
                            LRELU, bias=d["b1"][:, kt:kt + 1], alpha=SLOPE,
                        )

                # ---------- phase B: gather h_src, cast bf16, h_srcT ----------
                hs_bf = sbw.tile([128, nech, inf], BF16, tag="hsbf", name=f"hsbf{li}", bufs=1)
                hsT = sbw.tile([128, nit, e_pad], BF16, tag="hsT", name=f"hsT{li}", bufs=1)
                for c in range(nech):
                    hsg = sbw.tile([128, inf], F32, tag="hsg", name=f"hsg{li}", bufs=2)
                    nc.gpsimd.indirect_dma_start(
                        out=hsg[:], out_offset=None, in_=gather_src[:, :],
                        in_offset=bass.IndirectOffsetOnAxis(ap=srci_sb[:, c:c + 1], axis=0),
                    )
                    nc.vector.tensor_copy(hs_bf[:, c, :], hsg[:])
                    for it in range(nit):
                        p = min(128, inf - it * 128)
                        hstp = ps_wk.tile([128, 512], BF16, tag="wk", name=f"hstp{li}")
                        nc.tensor.transpose(
                            hstp[:p, :128],
                            hs_bf[:, c, it * 128:it * 128 + p], identb_sb[:],
                        )
                        nc.scalar.activation(
                            hsT[:p, it, c * 128:(c + 1) * 128], hstp[:p, :128], COPY,
                        )

                # msg scaled output tiles [128, nech*out] f32
                msg_sb = sbw.tile([128, nech, out], F32, tag="msg_sb", name=f"msg{li}", bufs=1)

                # ---------- phase C: big contraction per half ----------
                for hf in range(nhalf):
                    e0 = hf * 1024
                    msps = [
                        ps_msg.tile([128, 1024], F32, tag=f"msg{ot}", name=f"msps{li}_{ot}")
                        for ot in range(not_)
                    ]
                    # B2 bias term: msgT += B2[it].T @ h_srcT[it]
                    for ot in range(not_):
                        for q in range(2):
                            for it in range(nit):
                                p = min(128, inf - it * 128)
                                nc.tensor.matmul(
                                    msps[ot][:, q * 512:(q + 1) * 512],
                                    lhsT=d["b2"][:p, it, ot * 128:(ot + 1) * 128],
                                    rhs=hsT[:p, it, e0 + q * 512:e0 + (q + 1) * 512],
                                    start=(it == 0), stop=False,
                                    skip_group_check=True,
                                )
                    for i in range(inf):
                        bps = ps_b.tile([128, 1024], BF16, tag="bps", name=f"bps{li}")
                        for c8 in range(8):
                            c = hf * 8 + c8
                            nc.tensor.transpose(
                                bps[:, c8 * 128:(c8 + 1) * 128],
                                hs_bf[:, c, i:i + 1].to_broadcast([128, 128]),
                                identb_sb[:],
                            )
                        bsb = sbw.tile([128, 1024], BF16, tag="bsb", name=f"bsb{li}", bufs=2)
                        nc.scalar.activation(bsb[:], bps[:], COPY)
                        for kc in range(nkt):
                            zt = sbw.tile([128, 1024], BF16, tag="zt", name=f"zt{li}", bufs=3)
                            nc.vector.tensor_tensor(
                                out=zt[:], in0=ehT[:, kc, e0:e0 + 1024], in1=bsb[:],
                                op=mybir.AluOpType.mult,
                            )
                            w2t = sbw.tile([128, out], BF16, tag="w2t", name=f"w2t{li}", bufs=3)
                            r0 = (i * hid + kc * 128)
                            nc.sync.dma_start(w2t[:], din[f"w2p_{li}"][r0:r0 + 128, :])
                            last = (i == inf - 1) and (kc == nkt - 1)
                            for ot in range(not_):
                                for q in range(2):
                                    nc.tensor.matmul(
                                        msps[ot][:, q * 512:(q + 1) * 512],
                                        lhsT=w2t[:, ot * 128:(ot + 1) * 128],
                                        rhs=zt[:, q * 512:(q + 1) * 512],
                                        start=False, stop=last,
                                        skip_group_check=True,
                                    )
                    # ---------- phase D: msgT -> msg (transpose + rc scale) ----------
                    for ot in range(not_):
                        mts = sbw.tile([128, 1024], F32, tag="mts", name=f"mts{li}", bufs=2)
                        nc.scalar.activation(mts[:], msps[ot][:], COPY)
                        for c8 in range(8):
                            c = hf * 8 + c8
                            mtp = ps_wk.tile([128, 512], F32, tag="wk", name=f"mtp{li}")
                            nc.tensor.transpose(
                                mtp[:, :128], mts[:, c8 * 128:(c8 + 1) * 128], identf_sb[:],
                            )
                            nc.scalar.activation(
                                msg_sb[:, c, ot * 128:(ot + 1) * 128], mtp[:, :128],
                                COPY, scale=rcd_sb[:, c:c + 1],
                            )

                # hT_own for root term: transpose own h rows
                # layer0: own h = x_own; else hcat_tiles from previous layer
                hTo = sbw.tile([128, nit, V], F32, tag="hTo", name=f"hTo{li}", bufs=1)
                for vt in range(VT):
                    for it in range(nit):
                        p = min(128, inf - it * 128)
                        htp = ps_wk.tile([128, 512], F32, tag="wk", name=f"htp{li}")
                        if li == 0:
                            hsrc_own = sbw.tile([128, NODE_F], F32, tag="hso", name=f"hso{li}", bufs=2)
                            nc.sync.dma_start(
                                hsrc_own[:VTS, :], x_own[vt * VTS:(vt + 1) * VTS, :]
                            )
                            nc.tensor.transpose(
                                htp[:p, :VTS], hsrc_own[:VTS, it * 128:it * 128 + p],
                                identf_sb[:VTS, :VTS],
                            )
                        else:
                            nc.tensor.transpose(
                                htp[:p, :VTS],
                                hcat_tiles[vt][:VTS, it * 128:it * 128 + p],
                                identf_sb[:VTS, :VTS],
                            )
                        nc.scalar.activation(
                            hTo[:p, it, vt * VTS:(vt + 1) * VTS], htp[:p, :VTS], COPY,
                        )

                # ---------- phase E: aggregation + h' ----------
                new_hcat = []
                catw = out + NODE_F
                for vt in range(VT):
                    aggp = ps_wk.tile([128, 512], F32, tag="wk", name=f"aggp{li}")
                    for c in range(nech):
                        nc.tensor.matmul(
                            aggp[:VTS, :out],
                            lhsT=DT_sb[:, c * V + vt * VTS: c * V + (vt + 1) * VTS],
                            rhs=msg_sb[:, c, :],
                            start=(c == 0), stop=False,
                            skip_group_check=True,
                        )
                    for it in range(nit):
                        p = min(128, inf - it * 128)
                        nc.tensor.matmul(
                            aggp[:VTS, :out],
                            lhsT=hTo[:p, it, vt * VTS:(vt + 1) * VTS],
                            rhs=d["root"][:p, it, :],
                            start=False, stop=False,
                            skip_group_check=True,
                        )
                    nc.tensor.matmul(
                        aggp[:VTS, :out],
                        lhsT=ones_sb[:1, :VTS], rhs=d["biasrow"][:1, :],
                        start=False, stop=True,
                        skip_group_check=True,
                    )
                    hcat = sbw.tile([128, catw], F32, tag=f"hcat{li % 2}_{vt}", name=f"hcat{li}_{vt}", bufs=1)
                    nc.scalar.activation(hcat[:VTS, :out], aggp[:VTS, :out], LRELU, alpha=SLOPE)
                    nc.sync.dma_start(
                        hcat[:VTS, out:catw], x_own[vt * VTS:(vt + 1) * VTS, :]
                    )
                    new_hcat.append(hcat)
                hcat_tiles = new_hcat

                # ---------- phase F: write h_own + AllGather ----------
                if li < 2:
                    for vt in range(VT):
                        nc.sync.dma_start(
                            h_own[li][vt * VTS:(vt + 1) * VTS, :], hcat_tiles[vt][:VTS, :catw]
                        )
                    nc.gpsimd.collective_compute(
                        "AllGather", mybir.AluOpType.bypass,
                        replica_groups=RG,
                        ins=[h_own[li].opt()], outs=[h_full[li].opt()],
                    )

            # ---------- pooling ----------
            poolp = ps_wk.tile([128, 512], F32, tag="wk", name="poolp")
            for vt in range(VT):
                pt = sbw.tile([128, 128], F32, tag="pt", name="pt", bufs=2)
                bo = sbw.tile([128, 1], F32, tag="bo", name="bo", bufs=2)
                nc.sync.dma_start(bo[:VTS, :], batch_own[vt * VTS:(vt + 1) * VTS, :])
                nc.vector.tensor_scalar(
                    out=pt[:VTS, :], in0=iotaG_sb[:VTS, :],
                    scalar1=bo[:VTS, :1], scalar2=None,
                    op0=mybir.AluOpType.is_equal,
                )
                nc.tensor.matmul(
                    poolp[:N_GRAPHS, :F3],
                    lhsT=pt[:VTS, :N_GRAPHS], rhs=hcat_tiles[vt][:VTS, :F3],
                    start=(vt == 0), stop=(vt == VT - 1),
                    skip_group_check=True,
                )
            pool_sb = sbw.tile([128, F3], F32, name="pool_sb", bufs=1)
            nc.scalar.activation(pool_sb[:N_GRAPHS, :], poolp[:N_GRAPHS, :F3], COPY)
            nc.sync.dma_start(pool_in[:], pool_sb[:N_GRAPHS, :])
            nc.gpsimd.collective_compute(
                "AllReduce", mybir.AluOpType.add,
                replica_groups=RG,
                ins=[pool_in.opt()], outs=[pool_out.opt()],
            )
            pooled = sbw.tile([128, F3], F32, name="pooled", bufs=1)
            nc.sync.dma_start(pooled[:N_GRAPHS, :], pool_out[:])
            rcg_sb = sbw.tile([128, 1], F32, name="rcg_sb", bufs=1)
            nc.sync.dma_start(rcg_sb[:N_GRAPHS, :], rc_g[:])
            nc.vector.tensor_scalar_mul(pooled[:N_GRAPHS, :], pooled[:N_GRAPHS, :], rcg_sb[:N_GRAPHS, :1])

            # pooledT [F3 -> 3 chunks of 128, 128g]
            pooledT = sbw.tile([128, 3, 128], F32, name="pooledT", bufs=1)
            for it in range(3):
                p = min(128, F3 - it * 128)
                ptp = ps_wk.tile([128, 512], F32, tag="wk", name="ptp")
                nc.tensor.transpose(
                    ptp[:p, :N_GRAPHS], pooled[:N_GRAPHS, it * 128:it * 128 + p], identf_sb[:],
                )
                nc.scalar.activation(pooledT[:p, it, :], ptp[:p, :N_GRAPHS], COPY)

            # ---------- head ----------
            z1T = sbw.tile([128, 2, 128], F32, name="z1T", bufs=1)
            for mt in range(2):
                zp = ps_wk.tile([128, 512], F32, tag="wk", name="z1p")
                for it in range(3):
                    p = min(128, F3 - it * 128)
                    nc.tensor.matmul(
                        zp[:, :N_GRAPHS],
                        lhsT=fc1w_sb[:p, it, mt * 128:(mt + 1) * 128],
                        rhs=pooledT[:p, it, :],
                        start=(it == 0), stop=(it == 2),
                        skip_group_check=True,
                    )
                nc.scalar.activation(
                    z1T[:, mt, :], zp[:, :N_GRAPHS], LRELU,
                    bias=fc1b_sb[:, mt:mt + 1], alpha=SLOPE,
                )
            z2T = sbw.tile([128, 4, 128], F32, name="z2T", bufs=1)
            for mt in range(4):
                zp = ps_wk.tile([128, 512], F32, tag="wk", name="z2p")
                for it in range(2):
                    nc.tensor.matmul(
                        zp[:, :N_GRAPHS],
                        lhsT=fc2w_sb[:, it, mt * 128:(mt + 1) * 128],
                        rhs=z1T[:, it, :],
                        start=(it == 0), stop=(it == 1),
                        skip_group_check=True,
                    )
                nc.scalar.activation(
                    z2T[:, mt, :], zp[:, :N_GRAPHS], LRELU,
                    bias=fc2b_sb[:, mt:mt + 1], alpha=SLOPE,
                )
            yp = ps_wk.tile([128, 512], F32, tag="wk", name="yp")
            for it in range(4):
                nc.tensor.matmul(
                    yp[:1, :N_GRAPHS],
                    lhsT=fc3w_sb[:, it, :], rhs=z2T[:, it, :],
                    start=(it == 0), stop=False,
                    skip_group_check=True,
                )
            nc.tensor.matmul(
                yp[:1, :N_GRAPHS],
                lhsT=fc3b_sb[:1, :1], rhs=ones_sb[:1, :N_GRAPHS],
                start=False, stop=True,
                skip_group_check=True,
            )
            y_sb = sbw.tile([1, N_GRAPHS], F32, name="y_sb", bufs=1)
            nc.scalar.activation(y_sb[:], yp[:1, :N_GRAPHS], COPY)
            nc.sync.dma_start(y[:], y_sb[:])

    nc.compile()
    return nc


# ---------------- host side ----------------

_CACHE = {}


def _prep_inputs(x, edge_index, edge_attr, batch, weights):
    """Returns (e_pad, in_maps list of dicts)."""
    x = np.asarray(x, np.float32)
    ei = np.asarray(edge_index).astype(np.int64)
    ea = np.asarray(edge_attr, np.float32)
    batch = np.asarray(batch).astype(np.int64)

    src, dst = ei[0], ei[1]
    cnt = np.bincount(dst, minlength=N_NODES).astype(np.float32)
    rc_edge = 1.0 / np.maximum(cnt, 1.0)      # per dst node
    gcnt = np.bincount(batch, minlength=N_GRAPHS).astype(np.float32)
    rc_g = (1.0 / np.maximum(gcnt, 1.0)).reshape(N_GRAPHS, 1)

    bins = (dst // V).astype(np.int64)
    order = np.argsort(bins, kind="stable")
    counts = np.bincount(bins, minlength=N_CORES)
    e_pad = max(1024, int(np.ceil(counts.max() / 1024)) * 1024)

    iotaV = np.broadcast_to(np.arange(512, dtype=np.float32), (128, 512)).copy()
    iotaG = np.broadcast_to(np.arange(128, dtype=np.float32), (128, 128)).copy()
    ident_f = np.eye(128, dtype=np.float32)
    ident_bf = ident_f.astype(ml_dtypes.bfloat16)
    ones_row = np.ones((1, 128), np.float32)

    common = dict(
        x_full=x, rc_g=rc_g, ident_bf=ident_bf, ident_f=ident_f,
        iotaV=iotaV, iotaG=iotaG, ones_row=ones_row,
    )
    for li, (inf, hid, out) in enumerate(LAYERS):
        w1, b1, w2, b2, root, bias = weights[li]
        common[f"w1_{li}"] = np.asarray(w1, np.float32)
        common[f"b1_{li}"] = np.asarray(b1, np.float32).reshape(hid // 128, 128).T.copy()
        w2p = (
            np.asarray(w2, np.float32)
            .reshape(hid, inf, out).transpose(1, 0, 2).reshape(inf * hid, out)
        )
        common[f"w2p_{li}"] = w2p.astype(ml_dtypes.bfloat16)
        common[f"b2_{li}"] = np.asarray(b2, np.float32).reshape(inf, out).astype(ml_dtypes.bfloat16)
        common[f"root_{li}"] = np.asarray(root, np.float32)
        common[f"biasrow_{li}"] = np.asarray(bias, np.float32).reshape(1, out)
    fc1w, fc1b, fc2w, fc2b, fc3w, fc3b = weights[3]
    common["fc1w"] = np.asarray(fc1w, np.float32)
    common["fc1b"] = np.asarray(fc1b, np.float32).reshape(2, 128).T.copy()
    common["fc2w"] = np.asarray(fc2w, np.float32)
    common["fc2b"] = np.asarray(fc2b, np.float32).reshape(4, 128).T.copy()
    common["fc3w"] = np.asarray(fc3w, np.float32).reshape(512, 1)
    common["fc3b"] = np.asarray(fc3b, np.float32).reshape(1, 1)

    in_maps = []
    for c in range(N_CORES):
        sel = order[bins[order] == c]
        n = len(sel)
        srcp = np.zeros(e_pad, np.int32)
        dstlp = np.full(e_pad, -1.0, np.float32)
        rcp = np.zeros(e_pad, np.float32)
        eap = np.zeros((e_pad, EDGE_F), np.float32)
        srcp[:n] = src[sel]
        dstlp[:n] = (dst[sel] - c * V).astype(np.float32)
        rcp[:n] = rc_edge[dst[sel]]
        eap[:n] = ea[sel]
        m = dict(common)
        m["eaT"] = np.ascontiguousarray(eap.T)
        m["src_idx"] = srcp.reshape(e_pad, 1)
        m["dstl"] = dstlp.reshape(e_pad, 1)
        m["rc_dst"] = rcp.reshape(e_pad, 1)
        m["x_own"] = np.ascontiguousarray(x[c * V:(c + 1) * V])
        m["batch_own"] = batch[c * V:(c + 1) * V].astype(np.float32).reshape(V, 1)
        in_maps.append(m)
    return e_pad, in_maps


class _Runner:
    """Persistent PJRT executable for repeated timed runs."""

    def __init__(self, nc):
        import jax
        from jax.sharding import Mesh, PartitionSpec
        from jax.experimental.shard_map import shard_map
        from concourse import bass2jax

        bass2jax.install_neuronx_cc_hook()
        self.nc = nc
        partition_name = (
            nc.partition_id_tensor.name if nc.partition_id_tensor else None
        )
        in_names, out_names, out_avals, zero_outs = [], [], [], []
        for alloc in nc.m.functions[0].allocations:
            if not isinstance(alloc, mybir.MemoryLocationSet):
                continue
            name = alloc.memorylocations[0].name
            if alloc.kind == "ExternalInput":
                if name == partition_name:
                    continue
                in_names.append(name)
            elif alloc.kind == "ExternalOutput":
                out_names.append(name)
                shape = tuple(alloc.tensor_shape)
                dtype = mybir.dt.np(alloc.dtype)
                out_avals.append(jax.core.ShapedArray(shape, dtype))
                zero_outs.append(np.zeros(shape, dtype))
        self.in_names = in_names
        self.out_names = out_names
        self.zero_outs = zero_outs
        n_params = len(in_names)
        n_outs = len(out_names)
        all_names = in_names + out_names
        if partition_name is not None:
            all_names = all_names + [partition_name]
        donate = tuple(range(n_params, n_params + n_outs))

        def _body(*args):
            operands = list(args)
            if partition_name is not None:
                operands.append(bass2jax.partition_id_tensor())
            outs = bass2jax._bass_exec_p.bind(
                *operands,
                out_avals=tuple(out_avals),
                in_names=tuple(all_names),
                out_names=tuple(out_names),
                lowering_input_output_aliases=(),
                sim_require_finite=False,
                sim_require_nnan=False,
                nc=nc,
            )
            return tuple(outs)

        devices = jax.devices()[:N_CORES]
        mesh = Mesh(np.asarray(devices), ("core",))
        self.mesh = mesh
        in_specs = (PartitionSpec("core"),) * (n_params + n_outs)
        out_specs = (PartitionSpec("core"),) * n_outs
        self.fn = jax.jit(
            shard_map(_body, mesh=mesh, in_specs=in_specs,
                      out_specs=out_specs, check_rep=False),
            donate_argnums=donate, keep_unused=True,
        )
        self.out_avals = out_avals

    def put(self, in_maps):
        """Transfer inputs to device once; returns device buffers."""
        import jax
        from jax.sharding import NamedSharding, PartitionSpec
        sh = NamedSharding(self.mesh, PartitionSpec("core"))
        concat_in = [
            np.concatenate([np.asarray(in_maps[c][n]) for c in range(N_CORES)], axis=0)
            for n in self.in_names
        ]
        return [jax.device_put(a, sh) for a in concat_in]

    def run_dev(self, dev_in):
        """Run with device-resident inputs; fresh zero outputs (donated)."""
        import jax
        from jax.sharding import NamedSharding, PartitionSpec
        sh = NamedSharding(self.mesh, PartitionSpec("core"))
        concat_zero = [
            jax.device_put(
                np.zeros((N_CORES * z.shape[0], *z.shape[1:]), z.dtype), sh)
            for z in self.zero_outs
        ]
        out = self.fn(*dev_in, *concat_zero)
        jax.block_until_ready(out)
        return out

    def __call__(self, in_maps):
        out = self.run_dev(self.put(in_maps))
        return {
            n: np.asarray(out[i]).reshape(N_CORES, *self.out_avals[i].shape)[0]
            for i, n in enumerate(self.out_names)
        }


def _get_runner(e_pad):
    if e_pad not in _CACHE:
        nc = _build_nc(e_pad)
        _CACHE[e_pad] = _Runner(nc)
    return _CACHE[e_pad]


def kernel(x, edge_index, edge_attr, batch,
           m1w1, m1b1, m1w2, m1b2, root1, bias1,
           m2w1, m2b1, m2w2, m2b2, root2, bias2,
           m3w1, m3b1, m3w2, m3b2, root3, bias3,
           fc1w, fc1b, fc2w, fc2b, fc3w, fc3b):
    weights = [
        (m1w1, m1b1, m1w2, m1b2, root1, bias1),
        (m2w1, m2b1, m2w2, m2b2, root2, bias2),
        (m3w1, m3b1, m3w2, m3b2, root3, bias3),
        (fc1w, fc1b, fc2w, fc2b, fc3w, fc3b),
    ]
    e_pad, in_maps = _prep_inputs(x, edge_index, edge_attr, batch, weights)
    runner = _get_runner(e_pad)
    out = runner(in_maps)
    return np.ascontiguousarray(out["y"].reshape(N_GRAPHS, 1)).astype(np.float32)
